# revision 26
# baseline (speedup 1.0000x reference)
"""Multi-head attention (B=2, S=2048, D=1024, H=16) on 8 TRN2 NeuronCores.

Sharding: tensor-parallel over heads x data-parallel over batch.
Core c handles batch b = c // 4 and head group g = c % 4 (4 heads each).
Each core computes its 4 heads' q/k/v projections, attention, and the
partial output projection against its slice of Wo; the host sums the 4
partials per batch element.

Per-core kernel layout:
  - inputs: xT [1024, 2048] (= x[b].T), wq/wk/wv [1024, 256] (= W[rows].T),
    wo [256, 1024] (= Wo[:, cols].T)
  - QT/KT/VT computed transposed ([head-feat, seq]) so the Dh-contraction
    of q@k^T has its contraction dim on partitions.
  - scores are computed transposed ([keys, q]) for a head PAIR; the two
    K=64 score matmuls auto-derive row-tile positions (0,0)/(64,0) and run
    concurrently on the PE; one wide exp via ACT (scale folded); attn @ v
    contracts keys on partitions; columns 64:128 of the v operand hold
    ones so the same matmul emits softmax row-sums replicated across 64
    psum rows.

v4 structure. The scalar engine's exp stream is the critical resource
(128 exps x ~1.11us = 142.6us of ACT); the whole kernel is one flat
software-pipelined stream built to keep it dense:
  - attention is a single stream over all (pair, q-chunk, key-tile)
    groups; the attn@v matmuls LAG the score/exp stage by two groups so
    the PE never head-of-line blocks waiting for an exp result.
  - every non-attention PE task (projection chunks, V transposes, output
    projection tiles) is a small "filler" popped between groups, sized
    ~1-1.7us, scheduled 1-2 chunks before its consumer.
  - normalize runs entirely on the DVE (gpsimd is ~3x slower per op and
    back-pressured the PE through VA/OT dependencies when tried);
    softmax norms are emitted right after their chunk's last attn@v so
    the DVE never sits waiting at the head of its FIFO.
  - PE warmup matmuls use a DVE-memset f32 tile so the HAM un-throttles
    before the first projection; the ones block of the attn@v stationary
    is written by four broadcast DVE copies during the DMA head.
  - last chunk's normalize/output-projection/store pipeline is split
    128-columns-fine to shrink the serial tail.

fp16 streaming: matmul operands are fp16 (1 cyc/row on the PE vs ~1.6 for
f32r, half the LDWEIGHTS and SBUF cost) while every accumulation stays
f32 in PSUM. Value ranges fit fp16 comfortably (|q|,|k| ~ N(0,1),
exp(scores*scale) <= ~e^7; fp16 max is 65504).
"""

import numpy as np

B, S, D, H, DH = 2, 2048, 1024, 16, 64
NCORES = 8
GROUPS = 4  # head groups; 4 heads = 256 features per core
M = 256  # head features per core
SCALE = 0.125  # 1/sqrt(64)

# stream dtypes per matmul group: "f32r", "bf16", or "fp16"
CFG = {
    "proj": "fp16",   # xT, wq/wk/wv
    "scores": "fp16",  # QT, KT
    "av": "fp16",      # VA, exp tiles
    "wo": "fp16",      # OT, wo
    "out": "fp16",     # output staging + DMA; host sums partials in f32
}

_compiled = None


def _dt(mybir, name):
    return {"f32r": mybir.dt.float32r, "bf16": mybir.dt.bfloat16,
            "fp16": mybir.dt.float16, "f32": mybir.dt.float32}[name]


def _np_dt(name):
    if name == "bf16":
        import ml_dtypes
        return ml_dtypes.bfloat16
    if name == "fp16":
        return np.float16
    return np.float32


def _build_module():
    import concourse.mybir as mybir
    import concourse.tile as tile
    from concourse import bacc

    in_dt = _dt(mybir, CFG["proj"])
    wo_dt = _dt(mybir, CFG["wo"])
    out_dt = _dt(mybir, CFG["out"])
    nc = bacc.Bacc("TRN2", target_bir_lowering=False, debug=False,
                   num_devices=NCORES)
    xT = nc.dram_tensor("xT", [D, S], in_dt, kind="ExternalInput").ap()
    wq = nc.dram_tensor("wq", [D, M], in_dt, kind="ExternalInput").ap()
    wk = nc.dram_tensor("wk", [D, M], in_dt, kind="ExternalInput").ap()
    wv = nc.dram_tensor("wv", [D, M], in_dt, kind="ExternalInput").ap()
    wo = nc.dram_tensor("wo", [M, D], wo_dt, kind="ExternalInput").ap()
    out = nc.dram_tensor("out", [S, D], out_dt, kind="ExternalOutput").ap()

    with tile.TileContext(nc) as tc:
        _kernel_body(tc, out, xT, wq, wk, wv, wo)
    nc.compile()
    return nc


def _kernel_body(tc, out, xT, wq, wk, wv, wo):
    from contextlib import ExitStack

    import concourse.mybir as mybir
    from concourse.masks import make_identity

    nc = tc.nc
    f32 = mybir.dt.float32
    f32r = mybir.dt.float32r
    sc_dt = _dt(mybir, CFG["scores"])
    av_dt = _dt(mybir, CFG["av"])
    wo_dt = _dt(mybir, CFG["wo"])
    out_dt = _dt(mybir, CFG["out"])
    AF = mybir.ActivationFunctionType
    AL = mybir.AluOpType

    P = 128
    NKT = D // P   # 8 k-tiles in the projection contraction
    NPT = M // P   # 2 partition-tiles of head features
    SKT = S // P   # 16 key tiles
    QC = 512       # q chunk (psum bank width in f32)
    NQC = S // QC  # 4
    KPC = SKT // NQC  # 4 key tiles per x-chunk
    HPC = 4        # heads per core

    with ExitStack() as ctx:
        const = ctx.enter_context(tc.tile_pool(name="const", bufs=1))
        big = ctx.enter_context(tc.tile_pool(name="big", bufs=1))
        wpool = ctx.enter_context(tc.tile_pool(name="w", bufs=1))
        projin = ctx.enter_context(tc.tile_pool(name="projin", bufs=1))
        work = ctx.enter_context(tc.tile_pool(name="work", bufs=3))
        exp_pool = ctx.enter_context(tc.tile_pool(name="exp", bufs=12))
        small = ctx.enter_context(tc.tile_pool(name="small", bufs=2))
        # PSUM budget (8 banks): psA 2x1 + psS 2x2 + psO 2x1 = 8
        psum_big = ctx.enter_context(tc.tile_pool(name="psA", bufs=2, space="PSUM"))
        psum_s = ctx.enter_context(tc.tile_pool(name="psS", bufs=2, space="PSUM"))
        psum_o = ctx.enter_context(tc.tile_pool(name="psO", bufs=1, space="PSUM"))

        # ---- input DMAs first: weights (gpsimd queue) + xT chunks split
        # across the sync and scalar queues so chunk 0 lands ASAP ----
        w_sb = {}
        for name, w in (("k", wk), ("q", wq), ("v", wv)):
            t = projin.tile([P, NKT, M], w.dtype, tag=f"w{name}")
            nc.gpsimd.dma_start(t[:], w.rearrange("(kt p) m -> p kt m", p=P))
            w_sb[name] = t

        xT_sb = projin.tile([P, NKT, S], xT.dtype, tag="xT")
        xT_r = xT.rearrange("(kt p) s -> p kt s", p=P)
        for c in range(NQC):
            npieces = 4 if c == 0 else 2
            n = NKT // npieces
            for kh in range(npieces):
                eng = nc.sync if kh % 2 == 0 else nc.scalar
                eng.dma_start(
                    xT_sb[:, kh * n:(kh + 1) * n, c * QC:(c + 1) * QC],
                    xT_r[:, kh * n:(kh + 1) * n, c * QC:(c + 1) * QC])

        wo_sb = wpool.tile([P, NPT, D], wo_dt, tag="wo")
        nc.gpsimd.dma_start(wo_sb[:], wo.rearrange("(pt p) n -> p pt n", p=P))

        # ---- PE warmup: matmuls on a DVE-memset f32 tile (no gpsimd /
        # identity dependency) keep the HAM activity monitor busy through
        # the DMA head so projections start at 2.4GHz ----
        # wide (N=512) warmups keep the PE ~100% busy (small-N warmups are
        # LDWEIGHTS-paced at ~50% and the activity monitor never fires);
        # few enough that the first projection matmuls can start chasing
        # the xT DMA pieces as they land, which then keeps the PE busy
        # through the HAM window.
        warm_z = const.tile([P, 512], f32, tag="warm_z")
        nc.vector.memset(warm_z[:], 0.0)
        warm_h = const.tile([P, 512], av_dt, tag="warm_h")
        nc.vector.tensor_copy(warm_h[:], warm_z[:])
        warm_ps = psum_big.tile([P, 512], f32, tag="ps_big")
        for _ in range(5):
            nc.tensor.matmul(warm_ps[:], warm_h[:, 0:P], warm_h[:],
                             start=True, stop=True)

        ident_f = const.tile([P, P], f32)
        make_identity(nc, ident_f)
        ident = const.tile([P, P], f32r, tag="ident_r")
        nc.vector.tensor_copy(ident[:], ident_f[:])

        QT = big.tile([P, NPT, S], sc_dt, tag="QT")
        KT = big.tile([P, NPT, S], sc_dt, tag="KT")
        VT = big.tile([P, NPT, S], f32r, tag="VT")
        OT = big.tile([P, NPT, S], wo_dt, tag="OT")
        VA = big.tile([P, HPC, SKT, P], av_dt, tag="VA")

        # ones block (columns 64:128 of the attn@v stationary) -> row sums;
        # 16 plain DVE copies during the DMA head (a stride-0 broadcast
        # source compiles but produces garbage on real DVE hardware)
        ones32 = const.tile([P, 4, 64], f32, tag="ones32")
        nc.vector.memset(ones32[:], 1.0)
        for h in range(HPC):
            for st4 in range(0, SKT, 4):
                nc.vector.tensor_copy(VA[:, h, st4:st4 + 4, 64:128],
                                      ones32[:])

        # ---- building blocks ----
        def proj(name, pt, c):
            """PT[f, s] = sum_d w[d, f] * xT[d, s] for one 512-col chunk."""
            dst = {"q": QT, "k": KT, "v": VT}[name]
            ps = psum_big.tile([P, QC], f32, tag="ps_big")
            for kt in range(NKT):
                nc.tensor.matmul(
                    ps[:],
                    w_sb[name][:, kt, pt * P:(pt + 1) * P],
                    xT_sb[:, kt, c * QC:(c + 1) * QC],
                    start=(kt == 0), stop=(kt == NKT - 1),
                )
            nc.vector.tensor_copy(dst[:, pt, c * QC:(c + 1) * QC], ps[:])

        def vtrans(pt, c):
            """VT chunk back to natural layout in VA (values in cols 0:64)."""
            for st in range(KPC * c, KPC * (c + 1)):
                pst = psum_big.tile([P, P], f32r, tag="ps_big")
                nc.tensor.transpose(pst[:], VT[:, pt, st * P:(st + 1) * P],
                                    ident)
                nc.vector.tensor_copy(VA[:, 2 * pt, st, 0:64], pst[:, 0:64])
                nc.vector.tensor_copy(VA[:, 2 * pt + 1, st, 0:64],
                                      pst[:, 64:128])

        def norm(p, c, poA, poB):
            """softmax-normalize po into OT (all on DVE)."""
            cs = slice(c * QC, (c + 1) * QC)
            for r0, po in ((0, poA), (64, poB)):
                pc = small.tile([P, QC], f32, tag="po_sb")
                nc.vector.tensor_copy(pc[:], po[:])
                sm = small.tile([64, QC], f32, tag="sums")
                nc.vector.tensor_copy(sm[:], pc[64:128, :])
                rb = small.tile([64, QC], f32, tag="recip")
                nc.vector.reciprocal_approx_fast(rb[:], sm[:])
                nc.vector.tensor_tensor(OT[r0:r0 + 64, p, cs], pc[0:64, :],
                                        rb[:], AL.mult)

        def outproj_qt(qt, eng=None, act_cast=False):
            """output-projection partial + store for one 128-row q tile.
            act_cast routes the psum evacuation to the Scalar engine —
            only safe at the tail, when the exp stream has drained."""
            for nch in range(2):
                ps = psum_big.tile([P, 512], f32, tag="ps_big")
                for pt in range(NPT):
                    nc.tensor.matmul(
                        ps[:],
                        OT[:, pt, qt * P:(qt + 1) * P],
                        wo_sb[:, pt, nch * 512:(nch + 1) * 512],
                        start=(pt == 0), stop=(pt == NPT - 1),
                    )
                ot = work.tile([P, 512], out_dt, tag="outstage")
                if act_cast:
                    nc.scalar.copy(ot[:], ps[:])
                else:
                    nc.vector.tensor_copy(ot[:], ps[:])
                (eng or nc.sync).dma_start(
                    out[qt * P:(qt + 1) * P, nch * 512:(nch + 1) * 512],
                    ot[:])

        # ---- head: only k/q chunk 0 of pair 0 gate the first scores;
        # v/transpose ride as the first fillers (attn@v lags anyway) ----
        proj("k", 0, 0)
        proj("q", 0, 0)

        # ---- one flat attention stream over all (pair, chunk, key-tile)
        # groups; attn@v lags score/exp by AVLAG groups; fillers pop
        # between groups ----
        AVLAG = 4
        stream = [(p, c, kt) for p in range(NPT) for c in range(NQC)
                  for kt in range(SKT)]
        fillers = []
        # (emission index -> fillers to append) — every producer of a
        # filler's inputs is emitted before the filler can pop.
        filler_sched = {
            0: [lambda: proj("v", 0, 0), lambda: vtrans(0, 0),
                lambda: proj("k", 0, 1), lambda: proj("k", 0, 2),
                lambda: proj("v", 0, 1), lambda: vtrans(0, 1),
                lambda: proj("k", 0, 3), lambda: proj("v", 0, 2),
                lambda: vtrans(0, 2), lambda: proj("v", 0, 3),
                lambda: vtrans(0, 3), lambda: proj("q", 0, 1)],
            16: [lambda: proj("q", 0, 2), lambda: proj("k", 1, 0),
                 lambda: proj("k", 1, 1)],
            32: [lambda: proj("q", 0, 3), lambda: proj("k", 1, 2),
                 lambda: proj("k", 1, 3), lambda: proj("v", 1, 0),
                 lambda: vtrans(1, 0)],
            48: [lambda: proj("v", 1, 1), lambda: proj("q", 1, 0),
                 lambda: vtrans(1, 1), lambda: proj("v", 1, 2),
                 lambda: vtrans(1, 2), lambda: proj("v", 1, 3),
                 lambda: vtrans(1, 3)],
            64: [lambda: proj("q", 1, 1)],
            80: [lambda: proj("q", 1, 2)],
            96: [lambda: proj("q", 1, 3)],
        }

        po = {}
        ets = {}

        # Schraudolph exp on the DVE for 3 of 16 key tiles per chunk: the
        # fp16 bit pattern round(1024*log2e*scaled_score + 15360-58.5),
        # read back as fp16, approximates exp() to ~1.8% RMS. Offloading
        # these groups takes ~27us off the critical ACT stream; the
        # approximation error lands at ~8.5e-3 absmax-rel on the final
        # output (gate is 2e-2), validated by simulation.
        LOG2E = 1.4426950408889634
        SCHR_A = float(1024.0 * SCALE * LOG2E)
        SCHR_B = float(15360.0 - 58.5)
        SCHR_KTS = (6, 9, 12)

        def score_exp(g):
            p, c, kt = g
            cs = slice(c * QC, (c + 1) * QC)
            ks = slice(kt * P, (kt + 1) * P)
            pss = psum_s.tile([P, 2, QC], f32, tag="ps_s")
            nc.tensor.matmul(pss[:, 0, :], KT[0:64, p, ks],
                             QT[0:64, p, cs], start=True, stop=True)
            nc.tensor.matmul(pss[:, 1, :], KT[64:128, p, ks],
                             QT[64:128, p, cs], start=True, stop=True)
            et = exp_pool.tile([P, 2, QC], av_dt, tag="exp")
            if kt in SCHR_KTS:
                nc.vector.tensor_scalar(
                    et[:].bitcast(mybir.dt.int16), pss[:],
                    SCHR_A, SCHR_B, AL.mult, AL.add)
            else:
                nc.scalar.activation(et[:], pss[:], AF.Exp, scale=SCALE)
            ets[g] = et

        def av_A(g):
            p, c, kt = g
            if kt == 0:
                po[(p, c)] = (
                    psum_o.tile([P, QC], f32, tag="ps_oA", name="poA"),
                    psum_o.tile([P, QC], f32, tag="ps_oB", name="poB"))
            poA, _ = po[(p, c)]
            et = ets[g]
            nc.tensor.matmul(poA[:], VA[:, 2 * p, kt, :], et[:, 0, :],
                             start=(kt == 0), stop=(kt == SKT - 1))

        def av_B(g):
            p, c, kt = g
            _, poB = po[(p, c)]
            et = ets.pop(g)
            nc.tensor.matmul(poB[:], VA[:, 2 * p + 1, kt, :], et[:, 1, :],
                             start=(kt == 0), stop=(kt == SKT - 1))
            if kt == SKT - 1 and (p, c) != (1, NQC - 1):
                poA, _ = po[(p, c)]
                norm(p, c, poA, poB)
                if p == 1:  # chunk c normalized -> store it mid-next-chunk
                    # (popping it early would head-of-line block the PE on
                    # the not-yet-written OT columns)
                    key = 64 + 16 * (c + 1) + 8
                    filler_sched.setdefault(key, []).extend(
                        lambda qt=qt: outproj_qt(qt)
                        for qt in range(KPC * c, KPC * (c + 1)))

        # B lags one group behind A so a chunk-boundary psum-evacuation
        # wait stalls only one of the two accumulation streams at a time;
        # av_B (which carries the previous chunk's norm) is emitted before
        # av_A(kt0) so the norm reads its psum banks before reuse.
        for i, g in enumerate(stream):
            for f in filler_sched.pop(i, []):
                fillers.append(f)
            score_exp(g)
            if i >= AVLAG + 1:
                av_B(stream[i - AVLAG - 1])
            if i >= AVLAG:
                av_A(stream[i - AVLAG])
            if fillers and (i < 16 or i % 3 == 1):
                fillers.pop(0)()
        av_B(stream[-AVLAG - 1])
        for g in stream[-AVLAG:]:
            av_A(g)
            av_B(g)

        # ---- tail: last chunk normalized + projected + stored in 128-col
        # pieces so the serial tail is one piece, not one chunk ----
        # all DVE norm pieces first, then the output projections with their
        # psum casts on the (now idle) Scalar engine — the two chains run
        # on different engines instead of interleaving in one DVE FIFO
        c = NQC - 1
        poA, poB = po[(1, c)]
        for i, qt in enumerate(range(KPC * c, KPC * (c + 1))):
            qs = slice(i * P, (i + 1) * P)
            for r0, po_t in ((0, poA), (64, poB)):
                sm = small.tile([64, P], f32, tag="sums_t")
                nc.vector.tensor_copy(sm[:], po_t[64:128, qs])
                rb = small.tile([64, P], f32, tag="recip_t")
                nc.vector.reciprocal_approx_fast(rb[:], sm[:])
                nc.vector.tensor_tensor(
                    OT[r0:r0 + 64, 1, c * QC + i * P:c * QC + (i + 1) * P],
                    po_t[0:64, qs], rb[:], AL.mult)
        for i, qt in enumerate(range(KPC * c, KPC * (c + 1))):
            outproj_qt(qt, eng=(nc.sync if i % 2 == 0 else nc.gpsimd),
                       act_cast=True)

        while fillers:
            fillers.pop(0)()


def _in_maps(x, Wq, Wk, Wv, Wo):
    in_np = _np_dt(CFG["proj"])
    wo_np = _np_dt(CFG["wo"])
    x = np.asarray(x, dtype=np.float32)
    Wq = np.asarray(Wq, dtype=np.float32)
    Wk = np.asarray(Wk, dtype=np.float32)
    Wv = np.asarray(Wv, dtype=np.float32)
    Wo = np.asarray(Wo, dtype=np.float32)
    xT = [np.ascontiguousarray(x[b].T).astype(in_np) for b in range(B)]
    maps = []
    for c in range(NCORES):
        b, g = c // GROUPS, c % GROUPS
        rows = slice(g * M, (g + 1) * M)
        maps.append({
            "xT": xT[b],
            "wq": np.ascontiguousarray(Wq[rows, :].T).astype(in_np),
            "wk": np.ascontiguousarray(Wk[rows, :].T).astype(in_np),
            "wv": np.ascontiguousarray(Wv[rows, :].T).astype(in_np),
            "wo": np.ascontiguousarray(Wo[:, rows].T).astype(wo_np),
        })
    return maps


def kernel(x, Wq, Wk, Wv, Wo, _trace=False):
    global _compiled
    if _compiled is None:
        _compiled = _build_module()
    from concourse.bass_utils import run_bass_kernel_spmd

    res = run_bass_kernel_spmd(
        _compiled, _in_maps(x, Wq, Wk, Wv, Wo),
        core_ids=list(range(NCORES)), trace=_trace,
    )
    outs = [r["out"] for r in res.results]
    y = np.empty((B, S, D), np.float32)
    for b in range(B):
        y[b] = (outs[4 * b].astype(np.float32)
                + outs[4 * b + 1].astype(np.float32)
                + outs[4 * b + 2].astype(np.float32)
                + outs[4 * b + 3].astype(np.float32))
    if _trace:
        kernel.last_results = res
    return y


# revision 27
# speedup vs baseline: 1.0001x; 1.0001x over previous
"""Multi-head attention (B=2, S=2048, D=1024, H=16) on 8 TRN2 NeuronCores.

Sharding: tensor-parallel over heads x data-parallel over batch.
Core c handles batch b = c // 4 and head group g = c % 4 (4 heads each).
Each core computes its 4 heads' q/k/v projections, attention, and the
partial output projection against its slice of Wo; the host sums the 4
partials per batch element.

Per-core kernel layout:
  - inputs: xT [1024, 2048] (= x[b].T), wq/wk/wv [1024, 256] (= W[rows].T),
    wo [256, 1024] (= Wo[:, cols].T)
  - QT/KT/VT computed transposed ([head-feat, seq]) so the Dh-contraction
    of q@k^T has its contraction dim on partitions.
  - scores are computed transposed ([keys, q]) for a head PAIR; the two
    K=64 score matmuls auto-derive row-tile positions (0,0)/(64,0) and run
    concurrently on the PE; one wide exp via ACT (scale folded); attn @ v
    contracts keys on partitions; columns 64:128 of the v operand hold
    ones so the same matmul emits softmax row-sums replicated across 64
    psum rows.

v4 structure. The scalar engine's exp stream is the critical resource
(128 exps x ~1.11us = 142.6us of ACT); the whole kernel is one flat
software-pipelined stream built to keep it dense:
  - attention is a single stream over all (pair, q-chunk, key-tile)
    groups; the attn@v matmuls LAG the score/exp stage by two groups so
    the PE never head-of-line blocks waiting for an exp result.
  - every non-attention PE task (projection chunks, V transposes, output
    projection tiles) is a small "filler" popped between groups, sized
    ~1-1.7us, scheduled 1-2 chunks before its consumer.
  - normalize runs entirely on the DVE (gpsimd is ~3x slower per op and
    back-pressured the PE through VA/OT dependencies when tried);
    softmax norms are emitted right after their chunk's last attn@v so
    the DVE never sits waiting at the head of its FIFO.
  - PE warmup matmuls use a DVE-memset f32 tile so the HAM un-throttles
    before the first projection; the ones block of the attn@v stationary
    is written by four broadcast DVE copies during the DMA head.
  - last chunk's normalize/output-projection/store pipeline is split
    128-columns-fine to shrink the serial tail.

fp16 streaming: matmul operands are fp16 (1 cyc/row on the PE vs ~1.6 for
f32r, half the LDWEIGHTS and SBUF cost) while every accumulation stays
f32 in PSUM. Value ranges fit fp16 comfortably (|q|,|k| ~ N(0,1),
exp(scores*scale) <= ~e^7; fp16 max is 65504).
"""

import numpy as np

B, S, D, H, DH = 2, 2048, 1024, 16, 64
NCORES = 8
GROUPS = 4  # head groups; 4 heads = 256 features per core
M = 256  # head features per core
SCALE = 0.125  # 1/sqrt(64)

# stream dtypes per matmul group: "f32r", "bf16", or "fp16"
CFG = {
    "proj": "fp16",   # xT, wq/wk/wv
    "scores": "fp16",  # QT, KT
    "av": "fp16",      # VA, exp tiles
    "wo": "fp16",      # OT, wo
    "out": "fp16",     # output staging + DMA; host sums partials in f32
}

_compiled = None


def _dt(mybir, name):
    return {"f32r": mybir.dt.float32r, "bf16": mybir.dt.bfloat16,
            "fp16": mybir.dt.float16, "f32": mybir.dt.float32}[name]


def _np_dt(name):
    if name == "bf16":
        import ml_dtypes
        return ml_dtypes.bfloat16
    if name == "fp16":
        return np.float16
    return np.float32


def _build_module():
    import concourse.mybir as mybir
    import concourse.tile as tile
    from concourse import bacc

    in_dt = _dt(mybir, CFG["proj"])
    wo_dt = _dt(mybir, CFG["wo"])
    out_dt = _dt(mybir, CFG["out"])
    nc = bacc.Bacc("TRN2", target_bir_lowering=False, debug=False,
                   num_devices=NCORES)
    xT = nc.dram_tensor("xT", [D, S], in_dt, kind="ExternalInput").ap()
    wq = nc.dram_tensor("wq", [D, M], in_dt, kind="ExternalInput").ap()
    wk = nc.dram_tensor("wk", [D, M], in_dt, kind="ExternalInput").ap()
    wv = nc.dram_tensor("wv", [D, M], in_dt, kind="ExternalInput").ap()
    wo = nc.dram_tensor("wo", [M, D], wo_dt, kind="ExternalInput").ap()
    out = nc.dram_tensor("out", [S, D], out_dt, kind="ExternalOutput").ap()

    with tile.TileContext(nc) as tc:
        _kernel_body(tc, out, xT, wq, wk, wv, wo)
    nc.compile()
    return nc


def _kernel_body(tc, out, xT, wq, wk, wv, wo):
    from contextlib import ExitStack

    import concourse.mybir as mybir
    from concourse.masks import make_identity

    nc = tc.nc
    f32 = mybir.dt.float32
    f32r = mybir.dt.float32r
    sc_dt = _dt(mybir, CFG["scores"])
    av_dt = _dt(mybir, CFG["av"])
    wo_dt = _dt(mybir, CFG["wo"])
    out_dt = _dt(mybir, CFG["out"])
    AF = mybir.ActivationFunctionType
    AL = mybir.AluOpType

    P = 128
    NKT = D // P   # 8 k-tiles in the projection contraction
    NPT = M // P   # 2 partition-tiles of head features
    SKT = S // P   # 16 key tiles
    QC = 512       # q chunk (psum bank width in f32)
    NQC = S // QC  # 4
    KPC = SKT // NQC  # 4 key tiles per x-chunk
    HPC = 4        # heads per core

    with ExitStack() as ctx:
        const = ctx.enter_context(tc.tile_pool(name="const", bufs=1))
        big = ctx.enter_context(tc.tile_pool(name="big", bufs=1))
        wpool = ctx.enter_context(tc.tile_pool(name="w", bufs=1))
        projin = ctx.enter_context(tc.tile_pool(name="projin", bufs=1))
        work = ctx.enter_context(tc.tile_pool(name="work", bufs=3))
        exp_pool = ctx.enter_context(tc.tile_pool(name="exp", bufs=12))
        small = ctx.enter_context(tc.tile_pool(name="small", bufs=2))
        # PSUM budget (8 banks): psA 2x1 + psS 2x2 + psO 2x1 = 8
        psum_big = ctx.enter_context(tc.tile_pool(name="psA", bufs=2, space="PSUM"))
        psum_s = ctx.enter_context(tc.tile_pool(name="psS", bufs=2, space="PSUM"))
        psum_o = ctx.enter_context(tc.tile_pool(name="psO", bufs=1, space="PSUM"))

        # ---- input DMAs first: weights (gpsimd queue) + xT chunks split
        # across the sync and scalar queues so chunk 0 lands ASAP ----
        w_sb = {}
        for name, w in (("k", wk), ("q", wq), ("v", wv)):
            t = projin.tile([P, NKT, M], w.dtype, tag=f"w{name}")
            nc.gpsimd.dma_start(t[:], w.rearrange("(kt p) m -> p kt m", p=P))
            w_sb[name] = t

        xT_sb = projin.tile([P, NKT, S], xT.dtype, tag="xT")
        xT_r = xT.rearrange("(kt p) s -> p kt s", p=P)
        for c in range(NQC):
            npieces = 4 if c == 0 else 2
            n = NKT // npieces
            for kh in range(npieces):
                eng = nc.sync if kh % 2 == 0 else nc.scalar
                eng.dma_start(
                    xT_sb[:, kh * n:(kh + 1) * n, c * QC:(c + 1) * QC],
                    xT_r[:, kh * n:(kh + 1) * n, c * QC:(c + 1) * QC])

        wo_sb = wpool.tile([P, NPT, D], wo_dt, tag="wo")
        nc.gpsimd.dma_start(wo_sb[:], wo.rearrange("(pt p) n -> p pt n", p=P))

        # ---- PE warmup: matmuls on a DVE-memset f32 tile (no gpsimd /
        # identity dependency) keep the HAM activity monitor busy through
        # the DMA head so projections start at 2.4GHz ----
        # wide (N=512) warmups keep the PE ~100% busy (small-N warmups are
        # LDWEIGHTS-paced at ~50% and the activity monitor never fires);
        # few enough that the first projection matmuls can start chasing
        # the xT DMA pieces as they land, which then keeps the PE busy
        # through the HAM window.
        warm_z = const.tile([P, 512], f32, tag="warm_z")
        nc.vector.memset(warm_z[:], 0.0)
        warm_h = const.tile([P, 512], av_dt, tag="warm_h")
        nc.vector.tensor_copy(warm_h[:], warm_z[:])
        warm_ps = psum_big.tile([P, 512], f32, tag="ps_big")
        for _ in range(5):
            nc.tensor.matmul(warm_ps[:], warm_h[:, 0:P], warm_h[:],
                             start=True, stop=True)

        ident_f = const.tile([P, P], f32)
        make_identity(nc, ident_f)
        ident = const.tile([P, P], f32r, tag="ident_r")
        nc.vector.tensor_copy(ident[:], ident_f[:])

        QT = big.tile([P, NPT, S], sc_dt, tag="QT")
        KT = big.tile([P, NPT, S], sc_dt, tag="KT")
        VT = big.tile([P, NPT, S], f32r, tag="VT")
        OT = big.tile([P, NPT, S], wo_dt, tag="OT")
        VA = big.tile([P, HPC, SKT, P], av_dt, tag="VA")

        # ones block (columns 64:128 of the attn@v stationary) -> row sums;
        # 16 plain DVE copies during the DMA head (a stride-0 broadcast
        # source compiles but produces garbage on real DVE hardware)
        ones32 = const.tile([P, 4, 64], f32, tag="ones32")
        nc.vector.memset(ones32[:], 1.0)
        for h in range(HPC):
            for st4 in range(0, SKT, 4):
                nc.vector.tensor_copy(VA[:, h, st4:st4 + 4, 64:128],
                                      ones32[:])

        # ---- building blocks ----
        def proj(name, pt, c):
            """PT[f, s] = sum_d w[d, f] * xT[d, s] for one 512-col chunk."""
            dst = {"q": QT, "k": KT, "v": VT}[name]
            ps = psum_big.tile([P, QC], f32, tag="ps_big")
            for kt in range(NKT):
                nc.tensor.matmul(
                    ps[:],
                    w_sb[name][:, kt, pt * P:(pt + 1) * P],
                    xT_sb[:, kt, c * QC:(c + 1) * QC],
                    start=(kt == 0), stop=(kt == NKT - 1),
                )
            nc.vector.tensor_copy(dst[:, pt, c * QC:(c + 1) * QC], ps[:])

        def vtrans(pt, c):
            """VT chunk back to natural layout in VA (values in cols 0:64)."""
            for st in range(KPC * c, KPC * (c + 1)):
                pst = psum_big.tile([P, P], f32r, tag="ps_big")
                nc.tensor.transpose(pst[:], VT[:, pt, st * P:(st + 1) * P],
                                    ident)
                nc.vector.tensor_copy(VA[:, 2 * pt, st, 0:64], pst[:, 0:64])
                nc.vector.tensor_copy(VA[:, 2 * pt + 1, st, 0:64],
                                      pst[:, 64:128])

        def norm(p, c, poA, poB):
            """softmax-normalize po into OT (all on DVE)."""
            cs = slice(c * QC, (c + 1) * QC)
            for r0, po in ((0, poA), (64, poB)):
                pc = small.tile([P, QC], f32, tag="po_sb")
                nc.vector.tensor_copy(pc[:], po[:])
                sm = small.tile([64, QC], f32, tag="sums")
                nc.vector.tensor_copy(sm[:], pc[64:128, :])
                rb = small.tile([64, QC], f32, tag="recip")
                nc.vector.reciprocal_approx_fast(rb[:], sm[:])
                nc.vector.tensor_tensor(OT[r0:r0 + 64, p, cs], pc[0:64, :],
                                        rb[:], AL.mult)

        def outproj_qt(qt, eng=None):
            """output-projection partial + store for one 128-row q tile."""
            for nch in range(2):
                ps = psum_big.tile([P, 512], f32, tag="ps_big")
                for pt in range(NPT):
                    nc.tensor.matmul(
                        ps[:],
                        OT[:, pt, qt * P:(qt + 1) * P],
                        wo_sb[:, pt, nch * 512:(nch + 1) * 512],
                        start=(pt == 0), stop=(pt == NPT - 1),
                    )
                ot = work.tile([P, 512], out_dt, tag="outstage")
                nc.vector.tensor_copy(ot[:], ps[:])
                (eng or nc.sync).dma_start(
                    out[qt * P:(qt + 1) * P, nch * 512:(nch + 1) * 512],
                    ot[:])

        # ---- head: only k/q/v chunk 0 of pair 0 gate the first exp ----
        proj("k", 0, 0)
        proj("q", 0, 0)
        proj("v", 0, 0)
        vtrans(0, 0)

        # ---- one flat attention stream over all (pair, chunk, key-tile)
        # groups; attn@v lags score/exp by AVLAG groups; fillers pop
        # between groups ----
        AVLAG = 4
        stream = [(p, c, kt) for p in range(NPT) for c in range(NQC)
                  for kt in range(SKT)]
        fillers = []
        # (emission index -> fillers to append) — every producer of a
        # filler's inputs is emitted before the filler can pop.
        filler_sched = {
            0: [lambda: proj("k", 0, 1), lambda: proj("k", 0, 2),
                lambda: proj("v", 0, 1), lambda: vtrans(0, 1),
                lambda: proj("k", 0, 3), lambda: proj("v", 0, 2),
                lambda: vtrans(0, 2), lambda: proj("v", 0, 3),
                lambda: vtrans(0, 3), lambda: proj("q", 0, 1)],
            16: [lambda: proj("q", 0, 2), lambda: proj("k", 1, 0),
                 lambda: proj("k", 1, 1)],
            32: [lambda: proj("q", 0, 3), lambda: proj("k", 1, 2),
                 lambda: proj("k", 1, 3), lambda: proj("v", 1, 0),
                 lambda: vtrans(1, 0)],
            48: [lambda: proj("v", 1, 1), lambda: proj("q", 1, 0),
                 lambda: vtrans(1, 1), lambda: proj("v", 1, 2),
                 lambda: vtrans(1, 2), lambda: proj("v", 1, 3),
                 lambda: vtrans(1, 3)],
            64: [lambda: proj("q", 1, 1)],
            80: [lambda: proj("q", 1, 2)],
            96: [lambda: proj("q", 1, 3)],
        }

        po = {}
        ets = {}

        # Schraudolph exp on the DVE for 3 of 16 key tiles per chunk: the
        # fp16 bit pattern round(1024*log2e*scaled_score + 15360-58.5),
        # read back as fp16, approximates exp() to ~1.8% RMS. Offloading
        # these groups takes ~27us off the critical ACT stream; the
        # approximation error lands at ~8.5e-3 absmax-rel on the final
        # output (gate is 2e-2), validated by simulation.
        LOG2E = 1.4426950408889634
        SCHR_A = float(1024.0 * SCALE * LOG2E)
        SCHR_B = float(15360.0 - 58.5)
        SCHR_KTS = (6, 9, 12)

        def score_exp(g):
            p, c, kt = g
            cs = slice(c * QC, (c + 1) * QC)
            ks = slice(kt * P, (kt + 1) * P)
            pss = psum_s.tile([P, 2, QC], f32, tag="ps_s")
            nc.tensor.matmul(pss[:, 0, :], KT[0:64, p, ks],
                             QT[0:64, p, cs], start=True, stop=True)
            nc.tensor.matmul(pss[:, 1, :], KT[64:128, p, ks],
                             QT[64:128, p, cs], start=True, stop=True)
            et = exp_pool.tile([P, 2, QC], av_dt, tag="exp")
            if kt in SCHR_KTS:
                nc.vector.tensor_scalar(
                    et[:].bitcast(mybir.dt.int16), pss[:],
                    SCHR_A, SCHR_B, AL.mult, AL.add)
            else:
                nc.scalar.activation(et[:], pss[:], AF.Exp, scale=SCALE)
            ets[g] = et

        def av_A(g):
            p, c, kt = g
            if kt == 0:
                po[(p, c)] = (
                    psum_o.tile([P, QC], f32, tag="ps_oA", name="poA"),
                    psum_o.tile([P, QC], f32, tag="ps_oB", name="poB"))
            poA, _ = po[(p, c)]
            et = ets[g]
            nc.tensor.matmul(poA[:], VA[:, 2 * p, kt, :], et[:, 0, :],
                             start=(kt == 0), stop=(kt == SKT - 1))

        def av_B(g):
            p, c, kt = g
            _, poB = po[(p, c)]
            et = ets.pop(g)
            nc.tensor.matmul(poB[:], VA[:, 2 * p + 1, kt, :], et[:, 1, :],
                             start=(kt == 0), stop=(kt == SKT - 1))
            if kt == SKT - 1 and (p, c) != (1, NQC - 1):
                poA, _ = po[(p, c)]
                norm(p, c, poA, poB)
                if p == 1:  # chunk c normalized -> store it mid-next-chunk
                    # (popping it early would head-of-line block the PE on
                    # the not-yet-written OT columns)
                    key = 64 + 16 * (c + 1) + 8
                    filler_sched.setdefault(key, []).extend(
                        lambda qt=qt: outproj_qt(qt)
                        for qt in range(KPC * c, KPC * (c + 1)))

        # B lags one group behind A so a chunk-boundary psum-evacuation
        # wait stalls only one of the two accumulation streams at a time;
        # av_B (which carries the previous chunk's norm) is emitted before
        # av_A(kt0) so the norm reads its psum banks before reuse.
        for i, g in enumerate(stream):
            for f in filler_sched.pop(i, []):
                fillers.append(f)
            score_exp(g)
            if i >= AVLAG + 1:
                av_B(stream[i - AVLAG - 1])
            if i >= AVLAG:
                av_A(stream[i - AVLAG])
            if fillers and (i < 16 or i % 3 == 1):
                fillers.pop(0)()
        av_B(stream[-AVLAG - 1])
        for g in stream[-AVLAG:]:
            av_A(g)
            av_B(g)

        # ---- tail: last chunk normalized + projected + stored in 128-col
        # pieces so the serial tail is one piece, not one chunk ----
        c = NQC - 1
        poA, poB = po[(1, c)]
        for i, qt in enumerate(range(KPC * c, KPC * (c + 1))):
            qs = slice(i * P, (i + 1) * P)
            for r0, po_t in ((0, poA), (64, poB)):
                sm = small.tile([64, P], f32, tag="sums_t")
                nc.vector.tensor_copy(sm[:], po_t[64:128, qs])
                rb = small.tile([64, P], f32, tag="recip_t")
                nc.vector.reciprocal_approx_fast(rb[:], sm[:])
                nc.vector.tensor_tensor(
                    OT[r0:r0 + 64, 1, c * QC + i * P:c * QC + (i + 1) * P],
                    po_t[0:64, qs], rb[:], AL.mult)
            outproj_qt(qt, eng=(nc.sync if i % 2 == 0 else nc.scalar))

        while fillers:
            fillers.pop(0)()


def _in_maps(x, Wq, Wk, Wv, Wo):
    in_np = _np_dt(CFG["proj"])
    wo_np = _np_dt(CFG["wo"])
    x = np.asarray(x, dtype=np.float32)
    Wq = np.asarray(Wq, dtype=np.float32)
    Wk = np.asarray(Wk, dtype=np.float32)
    Wv = np.asarray(Wv, dtype=np.float32)
    Wo = np.asarray(Wo, dtype=np.float32)
    xT = [np.ascontiguousarray(x[b].T).astype(in_np) for b in range(B)]
    maps = []
    for c in range(NCORES):
        b, g = c // GROUPS, c % GROUPS
        rows = slice(g * M, (g + 1) * M)
        maps.append({
            "xT": xT[b],
            "wq": np.ascontiguousarray(Wq[rows, :].T).astype(in_np),
            "wk": np.ascontiguousarray(Wk[rows, :].T).astype(in_np),
            "wv": np.ascontiguousarray(Wv[rows, :].T).astype(in_np),
            "wo": np.ascontiguousarray(Wo[:, rows].T).astype(wo_np),
        })
    return maps


def kernel(x, Wq, Wk, Wv, Wo, _trace=False):
    global _compiled
    if _compiled is None:
        _compiled = _build_module()
    from concourse.bass_utils import run_bass_kernel_spmd

    res = run_bass_kernel_spmd(
        _compiled, _in_maps(x, Wq, Wk, Wv, Wo),
        core_ids=list(range(NCORES)), trace=_trace,
    )
    outs = [r["out"] for r in res.results]
    y = np.empty((B, S, D), np.float32)
    for b in range(B):
        y[b] = (outs[4 * b].astype(np.float32)
                + outs[4 * b + 1].astype(np.float32)
                + outs[4 * b + 2].astype(np.float32)
                + outs[4 * b + 3].astype(np.float32))
    if _trace:
        kernel.last_results = res
    return y
